# revision 1
# baseline (speedup 1.0000x reference)
"""Multi-head attention on 8 TRN2 NeuronCores (Bass/Tile).

Problem: B=4, S=2048, D=1024, H=16 heads (DH=64).
  out = softmax((q@wq+bq)(k@wk+bk)^T / sqrt(H)) @ (v@wv+bv) @ wo + bo

Sharding: 8 cores = 4 batches x 2 head-groups (8 heads each). Each core
computes its batch's QKV projections restricted to its head group's
columns, attention for those 8 heads, and a partial output projection
(wo rows for its heads); the host sums partials per batch. Activations
are kept TRANSPOSED ([feature, seq]) on device so every matmul has its
contraction on the partition dim with no on-device transposes; the host
transposes inputs/outputs (cheap numpy).

Per-core kernel (matmuls in float32r = full PE rate at free-dim>=256):
  phase A (DMA/PE balanced, streamed in 512-wide s-chunks):
    vh   = v^T-tiles^T @ wv          [2048, 512] natural, + ones col per
                                     head, stored bf16
    qhT  = wq^T @ qT + bq            [512, 2048]  (same for khT), f32r
  attention, per head h, per 1024-wide q window, per key tile (flat
  software-pipelined loop; ACT-bound at ~1.15us/step):
    scT  = khT_h-slice^T @ qhT_h     (K=64 contraction, PSUM [128,1024])
    eT   = exp(scT / 4)              (one ACT instr, PSUM->SBUF bf16)
    ctx_ext[65, 512] += vh_ext^T @ eT  (ones col -> row 64 = softmax sums)
    normalize: reciprocal(sums) (PSUM@p64 -> SBUF@p0), gpsimd
    partition_broadcast, DVE multiply -> ctxT (f32r)
  output projection wo^T @ ctxT is emitted INTO the attention's
  ACT-bound PE bubbles as background chains, gated on ctxT readiness:
  k-tiles {0,1} from head 4, {2} from head 6, {3, q-window 0} in the
  last window; only the k=3 remainder runs after attention. Partials
  are summed on the host (outTb in bf16 to halve the tail DMA).
Host: out[b] = (sum of partials of both cores).T + (bv @ wo + bo)
(bv/bo fold exactly through the linear tail since softmax rows sum to 1.)

The PV matmuls run with a 3-step software-pipeline skew behind the
exps so ctx-PSUM slot recycling never waits on the previous block's
normalize chain (reciprocal -> partition_broadcast -> multiply).
Depth 3 is the hardware-verified maximum: depth 4 races on silicon
(rel err 0.78) despite simulating fine.

Measured (8-core SPMD, axon): rel err vs fp32 reference 2.3e-3;
cost-model timeline ~421.6 us/core (ACT-exp stream ~277us fully
packed; ~110us DMA-bound load+projection phase A + ~32us tail).
"""
import ml_dtypes
import numpy as np

import concourse.bass as bass
import concourse.mybir as mybir
from concourse import bacc
from concourse.tile import TileContext
from concourse.bass_utils import run_bass_kernel_spmd

B, S, D, H = 4, 2048, 1024, 16
DH = D // H          # 64
HG = H // 2          # 8 heads per core
DL = HG * DH         # 512 local qkv width
NG = 2               # head subgroups per core (SBUF fit)
HSUB = HG // NG      # 4 heads per subgroup
DSUB = HSUB * DH     # 256 columns per subgroup
KT = D // 128        # 8 contraction tiles for projections
ST = S // 128        # 16 key tiles
QW = S // 1024       # 2 q windows of 1024
SCALE = 1.0 / np.sqrt(np.float32(H))  # 0.25

f32 = mybir.dt.float32
f32r = mybir.dt.float32r
bf16 = mybir.dt.bfloat16


def _build_program() -> bacc.Bacc:
    nc = bacc.Bacc()
    qT_e = nc.declare_dram_parameter("qT", [D, S], f32r, isOutput=False)
    kT_e = nc.declare_dram_parameter("kT", [D, S], f32r, isOutput=False)
    vT_e = nc.declare_dram_parameter("vT", [D, S], f32r, isOutput=False)
    wq_e = nc.declare_dram_parameter("wq", [D, DL], f32r, isOutput=False)
    wk_e = nc.declare_dram_parameter("wk", [D, DL], f32r, isOutput=False)
    wv_e = nc.declare_dram_parameter("wv", [D, DL], f32r, isOutput=False)
    wo_e = nc.declare_dram_parameter("wo", [DL, D], f32r, isOutput=False)
    bq_e = nc.declare_dram_parameter("bq", [DL], f32, isOutput=False)
    bk_e = nc.declare_dram_parameter("bk", [DL], f32, isOutput=False)
    outa_e = nc.declare_dram_parameter("outTa", [D, S], f32, isOutput=True)
    outb_e = nc.declare_dram_parameter("outTb", [D, S], mybir.dt.bfloat16,
                                       isOutput=True)
    outc_e = nc.declare_dram_parameter("outTc", [D, S], f32, isOutput=True)

    with TileContext(nc) as tc:
        with (
            tc.tile_pool(name="wp", bufs=1) as wpool,
            tc.tile_pool(name="inp", bufs=3) as inpool,
            tc.tile_pool(name="proj", bufs=1) as projpool,
            tc.tile_pool(name="attn", bufs=1) as attnpool,
            tc.tile_pool(name="sm", bufs=3) as smpool,
            tc.tile_pool(name="ps", bufs=2, space="PSUM") as pspool,
            tc.tile_pool(name="ctxps", bufs=3, space="PSUM") as ctxpspool,
            tc.tile_pool(name="bgps", bufs=1, space="PSUM") as bgpspool,
        ):
            # ---- constants: wo + biases (live whole kernel) ----
            wo_t = [
                wpool.tile([128, D], f32r, tag="wo", bufs=4, name=f"wo{t}")
                for t in range(4)
            ]
            for t in range(4):
                nc.sync.dma_start(out=wo_t[t], in_=wo_e[t * 128:(t + 1) * 128, :])
            # bias tile [128, DL//128]: column j holds b[128j:128j+128]
            bias_t = {}
            for nm, ext in [("bq", bq_e), ("bk", bk_e)]:
                bias_t[nm] = wpool.tile([128, DL // 128], f32, tag=nm, name=f"b_{nm}")
                nc.sync.dma_start(
                    out=bias_t[nm], in_=ext.rearrange("(j p) -> p j", p=128)
                )

            # ctxT: heads stacked on partitions, 4 tiles of [128, S]
            ctxT = [
                attnpool.tile([128, S], f32r, tag="ctxT", bufs=4, name=f"ctxT{t}")
                for t in range(4)
            ]

            def load_wproj(nm, ext, dt_=f32r):
                # full-width weight tiles [128, DL]; one shared slot ring
                # (phases are sequential, slots recycle across q/k/v)
                wt = []
                for t in range(KT):
                    w = wpool.tile([128, DL], dt_, tag="wproj", bufs=KT,
                                   name=f"w{nm}t{t}")
                    nc.sync.dma_start(out=w, in_=ext[t * 128:(t + 1) * 128, :])
                    wt.append(w)
                return wt

            def make_vh():
                # vh natural for ALL 8 heads + ones cols: 16 tiles [128, 8*65]
                wt = load_wproj("wv", wv_e)
                tiles = []
                for sc in range(4):
                    vxt = []
                    for t in range(KT):
                        x = inpool.tile([128, 512], f32r, tag="in", bufs=16,
                                        name=f"vTs{sc}t{t}")
                        nc.sync.dma_start(
                            out=x,
                            in_=vT_e[t * 128:(t + 1) * 128,
                                     sc * 512:(sc + 1) * 512],
                        )
                        vxt.append(x)
                    for sti in range(4):
                        st = sc * 4 + sti
                        ps = pspool.tile([128, DL], f32, tag="sc", bufs=2,
                                         name=f"vps{st}")
                        for t in range(KT):
                            nc.tensor.matmul(
                                ps[:, :],
                                vxt[t][:, sti * 128:(sti + 1) * 128],
                                wt[t][:, :],
                                start=(t == 0), stop=(t == KT - 1),
                            )
                        vt = projpool.tile([128, HG, 65], bf16, tag="vh", bufs=ST,
                                           name=f"vh{st}")
                        nc.vector.tensor_copy(
                            vt[:, :, 0:64], ps.rearrange("p (h d) -> p h d", h=HG)
                        )
                        nc.vector.memset(vt[:, :, 64:65], 1.0)
                        tiles.append(vt)
                return tiles

            def make_proj(nm, wext, xname, xext, bname, ms=(0, 1, 2, 3)):
                # full-width projection [DL, S] = 4 tiles [128, S].
                # ms restricts which dh-tiles are computed now; the rest can
                # be filled later by proj_bg (weights tiles are returned).
                wt = load_wproj(nm, wext)
                tiles = [
                    projpool.tile([128, S], f32r, tag=nm, bufs=4,
                                  name=f"{nm}m{m}")
                    for m in range(4)
                ]
                for sc in range(4):  # 512-wide s chunks
                    xt = []
                    for t in range(KT):
                        x = inpool.tile([128, 512], f32r, tag="in", bufs=16,
                                        name=f"{xname}s{sc}t{t}")
                        nc.sync.dma_start(
                            out=x,
                            in_=xext[t * 128:(t + 1) * 128,
                                     sc * 512:(sc + 1) * 512],
                        )
                        xt.append(x)
                    for m in ms:  # dh tile
                        ps = pspool.tile([128, 512], f32, tag="sc", bufs=2,
                                         name=f"pps{nm}{m}{sc}")
                        for t in range(KT):
                            nc.tensor.matmul(
                                ps[:, :],
                                wt[t][:, m * 128:(m + 1) * 128],
                                xt[t][:, :],
                                start=(t == 0), stop=(t == KT - 1),
                            )
                        nc.vector.tensor_scalar_add(
                            tiles[m][:, sc * 512:(sc + 1) * 512], ps[:, :],
                            bias_t[bname][:, m:m + 1],
                        )
                return tiles, wt

            def proj_bg(nm, tiles, wt, xname, xext, bname, ms):
                # background completion of deferred dh-tiles: reload x
                # (DMA is idle during attention) and run the chains in the
                # spare "bg" PSUM bank, one MM per attention step.
                for sc in range(4):
                    xt = []

                    def emit_xdma(t, sc=sc, xt=xt):
                        def go():
                            x = inpool.tile([128, 512], f32r, tag="in",
                                            bufs=16, name=f"{xname}b{sc}t{t}")
                            nc.sync.dma_start(
                                out=x,
                                in_=xext[t * 128:(t + 1) * 128,
                                         sc * 512:(sc + 1) * 512],
                            )
                            xt.append(x)
                        return go
                    for t in range(KT):
                        yield ("dma", emit_xdma(t))
                    for m in ms:
                        state = {}

                        def emit_mm(t, m=m, sc=sc, xt=xt, state=state):
                            def go():
                                if t == 0:
                                    state["ps"] = bgpspool.tile(
                                        [128, 512], f32, tag="bg", bufs=1,
                                        name=f"bps{nm}{m}{sc}")
                                nc.tensor.matmul(
                                    state["ps"][:, :],
                                    wt[t][:, m * 128:(m + 1) * 128],
                                    xt[t][:, :],
                                    start=(t == 0), stop=(t == KT - 1),
                                )
                                if t == KT - 1:
                                    nc.vector.tensor_scalar_add(
                                        tiles[m][:, sc * 512:(sc + 1) * 512],
                                        state["ps"][:, :],
                                        bias_t[bname][:, m:m + 1],
                                    )
                            return go
                        for t in range(KT):
                            yield ("mm", emit_mm(t))

            # ---- background: wo partial chains (k-tiles 0..2) ----
            ota_state = {"n": 0}

            def wo_chain(m, sc, kts, out_ext, tag, use_act=False):
                # one output-projection chain over the given ctxT k-tiles
                if tag in ("a", "c"):
                    ps = bgpspool.tile([128, 512], f32, tag="bg", bufs=1,
                                       name=f"bg{tag}{m}{sc}")
                else:
                    ps = ctxpspool.tile([128, 512], f32, tag="ctx", bufs=3,
                                        name=f"bg{tag}{m}{sc}")
                for i, t in enumerate(kts):
                    yield ("mm", lambda t=t, i=i, ps=ps: nc.tensor.matmul(
                        ps[:, :],
                        wo_t[t][:, m * 128:(m + 1) * 128],
                        ctxT[t][:, sc * 512:(sc + 1) * 512],
                        start=(i == 0), stop=(i == len(kts) - 1),
                    ))

                def drain(ps=ps, m=m, sc=sc):
                    dt_ = bf16 if tag == "b" else f32
                    ot = smpool.tile([128, 512], dt_, tag="ot" + tag,
                                     bufs=2 if tag == "a" else 3,
                                     name=f"ot{tag}{m}{sc}")
                    if use_act:
                        nc.scalar.copy(ot[:, :], ps[:, :])
                    else:
                        nc.vector.tensor_copy(ot[:, :], ps[:, :])
                    nc.sync.dma_start(
                        out=out_ext[m * 128:(m + 1) * 128,
                                    sc * 512:(sc + 1) * 512],
                        in_=ot[:, :],
                    )
                yield ("dma", drain)

            def bg_streams():
                # (from_hl, from_qw, generator): wo partials paced by ctxT
                # readiness (ctxT[0,1] after hl3, ctxT[2] after hl5, ctxT[3]
                # q-window 0 after hl7/qw0).
                def gen(kts, ext, tag, scs=(0, 1, 2, 3)):
                    for m in range(8):
                        for sc in scs:
                            yield from wo_chain(m, sc, kts, ext, tag)
                # last field: steps to skip before first pop (lets the
                # gating block's own normalize land first)
                return [[4, 0, gen([0, 1], outa_e, "a"), 0],
                        [6, 0, gen([2], outc_e, "c"), 0],
                        [7, 1, gen([3], outb_e, "b", scs=(0, 1)), 5]]

            def attention(vh_tiles, qhT, khT, bgs=()):
                pending_pv = []  # (emit_fn, finalize_or_None)

                def flush_one_pv(depth=3):
                    if len(pending_pv) >= depth:
                        emit, fin = pending_pv.pop(0)
                        emit()
                        if fin is not None:
                            fin()

                def pop_bg(hl, qw):
                    for ent in bgs:
                        if (hl, qw) < (ent[0], ent[1]) or ent[2] is None:
                            continue
                        if ent[3] > 0:
                            ent[3] -= 1
                            continue
                        while True:
                            try:
                                kind, go = next(ent[2])
                            except StopIteration:
                                ent[2] = None
                                break
                            go()
                            if kind == "mm":
                                return

                def make_finalize(hl, qw, ctx_ps, ct_tile, hb):
                    def fin():
                        # normalize per 512 chunk: recip of sums row
                        # (psum@base64 -> sbuf@base0), broadcast, multiply
                        for c in range(2):
                            qoff = qw * 1024 + c * 512
                            rc = smpool.tile([1, 512], f32, tag="rc", bufs=1,
                                             name=f"rc{hl}{qw}{c}")
                            nc.vector.reciprocal(rc[0:1, :], ctx_ps[c][64:65, :])
                            rb = smpool.tile([64, 512], f32, tag="rb", bufs=1,
                                             name=f"rb{hl}{qw}{c}")
                            nc.gpsimd.partition_broadcast(rb[:, :], rc[0:1, :])
                            if hb == 0:
                                nc.vector.tensor_mul(
                                    ct_tile[0:64, qoff:qoff + 512],
                                    ctx_ps[c][0:64, :], rb[:, :],
                                )
                            else:
                                stg = smpool.tile([64, 512], f32r, tag="stg",
                                                  bufs=2, name=f"stg{hl}{qw}{c}")
                                nc.vector.tensor_mul(stg[:, :], ctx_ps[c][0:64, :],
                                                     rb[:, :])
                                nc.gpsimd.dma_start(
                                    out=ct_tile[hb:hb + 64, qoff:qoff + 512],
                                    in_=stg[:, :],
                                )
                    return fin

                for hl in range(HG):
                    qh_tile = qhT[hl // 2]
                    kh_tile = khT[hl // 2]
                    hb = (hl % 2) * 64
                    ct_tile = ctxT[hl // 2]
                    for qw in range(QW):
                        ctx_ps = [
                            ctxpspool.tile([65, 512], f32, tag="ctx", bufs=3,
                                           name=f"ctx{hl}{qw}{c}")
                            for c in range(2)
                        ]
                        for st in range(ST):
                            sc_ps = pspool.tile(
                                [128, 1024], f32, tag="sc", bufs=2,
                                name=f"sc{hl}{qw}{st}",
                            )
                            for half in range(2):
                                nc.tensor.matmul(
                                    sc_ps[:, half * 512:(half + 1) * 512],
                                    kh_tile[hb:hb + 64, st * 128:(st + 1) * 128],
                                    qh_tile[hb:hb + 64,
                                            qw * 1024 + half * 512:
                                            qw * 1024 + (half + 1) * 512],
                                    start=True, stop=True,
                                )
                            et = smpool.tile(
                                [128, 1024], bf16, tag="expT", bufs=4,
                                name=f"et{hl}{qw}{st}",
                            )
                            nc.scalar.activation(
                                et[:, :], sc_ps[:, :],
                                mybir.ActivationFunctionType.Exp,
                                scale=float(SCALE),
                            )
                            flush_one_pv()
                            pop_bg(hl, qw)

                            def make_pv(st=st, et=et, ctx_ps=ctx_ps,
                                        vt=vh_tiles[st], hl=hl):
                                def emit():
                                    for half in range(2):
                                        nc.tensor.matmul(
                                            ctx_ps[half][:, :],
                                            vt[:, hl, :],
                                            et[:, half * 512:(half + 1) * 512],
                                            start=(st == 0), stop=(st == ST - 1),
                                        )
                                return emit
                            fin = (make_finalize(hl, qw, ctx_ps, ct_tile, hb)
                                   if st == ST - 1 else None)
                            pending_pv.append((make_pv(), fin))
                while pending_pv:
                    flush_one_pv(depth=1)

            # ---- schedule ----
            vh_tiles = make_vh()
            qhT, _ = make_proj("qhT", wq_e, "qT", qT_e, "bq")
            khT, _ = make_proj("khT", wk_e, "kT", kT_e, "bk")
            bgs = bg_streams()
            attention(vh_tiles, qhT, khT, bgs=bgs)
            # leftover background (if any) + rest of the k=3 partial (bf16)
            for ent in bgs:
                if ent[2] is not None:
                    for kind, go in ent[2]:
                        go()
            for m in range(8):
                for sc in (2, 3):
                    for kind, go in wo_chain(m, sc, [3], outb_e, "b",
                                             use_act=(sc % 2 == 0)):
                        go()

    nc.compile()
    return nc


_NC = None


def _get_program():
    global _NC
    if _NC is None:
        _NC = _build_program()
    return _NC


def make_in_maps(q, k, v, wq, wk, wv, wo, bq, bk):
    in_maps = []
    for b in range(B):
        qT = np.ascontiguousarray(q[b].T)
        kT = np.ascontiguousarray(k[b].T)
        vT = np.ascontiguousarray(v[b].T)
        for g in range(2):
            cols = slice(g * DL, (g + 1) * DL)
            in_maps.append({
                "qT": qT, "kT": kT, "vT": vT,
                "wq": np.ascontiguousarray(wq[:, cols]),
                "wk": np.ascontiguousarray(wk[:, cols]),
                "wv": np.ascontiguousarray(wv[:, cols]),
                "wo": np.ascontiguousarray(wo[cols, :]),
                "bq": np.ascontiguousarray(bq[cols]),
                "bk": np.ascontiguousarray(bk[cols]),
            })
    return in_maps


def assemble_out(results, wo, bv, bo):
    tail = bv @ wo + bo  # exact fold of v/output biases (softmax rows sum to 1)
    out = np.empty((B, S, D), np.float32)
    for b in range(B):
        acc = sum(
            results[2 * b + g][k].astype(np.float32)
            for g in range(2) for k in ("outTa", "outTb", "outTc")
        )
        out[b] = acc.T + tail
    return out


def kernel(q, k, v, wq, bq, wk, bk, wv, bv, wo, bo, **_unused):
    q = np.asarray(q, np.float32)
    k = np.asarray(k, np.float32)
    v = np.asarray(v, np.float32)
    wq = np.asarray(wq, np.float32)
    wk = np.asarray(wk, np.float32)
    wv = np.asarray(wv, np.float32)
    wo = np.asarray(wo, np.float32)
    bq = np.asarray(bq, np.float32)
    bk = np.asarray(bk, np.float32)
    bv = np.asarray(bv, np.float32)
    bo = np.asarray(bo, np.float32)

    nc = _get_program()
    in_maps = make_in_maps(q, k, v, wq, wk, wv, wo, bq, bk)
    res = run_bass_kernel_spmd(nc, in_maps, core_ids=list(range(8))).results
    return assemble_out(res, wo, bv, bo)


if __name__ == "__main__":
    rng = np.random.default_rng(0)
    sd = 1.0 / np.sqrt(D)
    inputs = {
        "q": rng.standard_normal((B, S, D), dtype=np.float32),
        "k": rng.standard_normal((B, S, D), dtype=np.float32),
        "v": rng.standard_normal((B, S, D), dtype=np.float32),
        "wq": rng.standard_normal((D, D), dtype=np.float32) * sd,
        "bq": np.zeros(D, np.float32),
        "wk": rng.standard_normal((D, D), dtype=np.float32) * sd,
        "bk": np.zeros(D, np.float32),
        "wv": rng.standard_normal((D, D), dtype=np.float32) * sd,
        "bv": np.zeros(D, np.float32),
        "wo": rng.standard_normal((D, D), dtype=np.float32) * sd,
        "bo": np.zeros(D, np.float32),
    }
    out = kernel(**inputs)
    print("kernel ran:", out.shape, out.dtype)

